# revision 1
# baseline (speedup 1.0000x reference)
"""Trainium2 Bass kernel for nn_EnvEncoder (7-branch MLP + 2x LayerNorm).

Contract: kernel(**inputs) takes the FULL unsharded inputs (x: [524288, 94] f32
plus small weights) and returns the FULL output [524288, 128] f32.

Strategy (pure data parallel over 8 cores, 65536 rows/core):
  - Host: fold the 7 branch Linears into one block-diagonal W1 [95, 160]
    (row 94 = concatenated biases; x is transposed and augmented with a ones
    row on the host so mm1 = xT_aug.T @ W1 includes the bias).
    W2 is w_fuse with row-centered columns (so LN2 mean-subtract is exact and
    free) + a bias row (centered b_fuse), consumed via a ones column in u.
  - Device, per 128-sample tile (row-major: samples on partitions):
      mm1 (PE) -> relu (ACT, fused PSUM->SBUF, batched over 3 tiles)
      -> LN1 mean/var via grouped bn_stats + bn_aggr (DVE)
      -> fused (h-mu)*rstd via dual-op tensor_scalar (DVE)
      -> PE transpose of u[128,161] (two chunks) -> relu-folded PSUM->SBUF
      -> mm2 (PE, 2 accumulating matmuls) -> LN2 var via grouped bn_stats
      -> final relu(h2c * rstd2) (per-partition scale) -> DMA out.
    rstd math (reciprocal + sqrt) is batched across tiles.
"""

import os
import numpy as np
import ml_dtypes

import concourse.bass as bass
import concourse.bacc as bacc
import concourse.tile as tile
from concourse import mybir
from concourse.bass_utils import run_bass_kernel_spmd

B_TOTAL = 524288
N_CORES = 8
B_CORE = B_TOTAL // N_CORES  # 65536
P = 128                       # samples per tile (partition dim)
K1 = 95                       # 94 features + ones row
F1 = 160                      # hidden features
F1A = 128                     # first transpose chunk
# second transpose reads u[:, 33:161] (full 128 cols so its PSUM output is
# fully initialized); mm2b contracts its rows 64:128 (= u cols 97:160 + ones)
# against a zero-padded W2b [64, 128] whose first 31 rows are zero.
F1B = 64                      # mm2b contraction size
T2_LO = 33                    # u column where the second transpose starts
F2 = 128                      # output features
SG = 12                       # tiles per supergroup (LN1 stat batching)
G1 = 3                        # mm1 outputs per PSUM bank tile
G2 = 3                        # mm2 outputs per PSUM bank tile
LN2_BATCH = 6                 # tiles per LN2 rstd batch
EPS = 1e-5

# Branch layout: (in_lo, in_hi, out_lo, out_hi)
_BRANCHES = [
    ("month", 0, 12, 0, 32),
    ("area", 12, 18, 32, 48),
    ("icls", 18, 24, 48, 64),
    ("scalar", 24, 26, 64, 80),
    ("long", 26, 62, 80, 112),
    ("lat", 62, 74, 112, 128),
    ("hist", 74, 94, 128, 160),
]

# Compute dtype for matmul operands / activations ("float32" or "bfloat16").
DT_NAME = os.environ.get("ENVENC_DT", "bfloat16")
TRACE = False  # set by test harness for profiled runs

_PROGRAM_CACHE = {}
LAST_RESULTS = None  # BassKernelResults of the most recent run


def _np_dt(dt_name):
    return np.float32 if dt_name == "float32" else ml_dtypes.bfloat16


def _my_dt(dt_name):
    return mybir.dt.float32 if dt_name == "float32" else mybir.dt.bfloat16


def _iter_chunks(n, size):
    out = []
    i = 0
    while i < n:
        out.append((i, min(size, n - i)))
        i += size
    return out


def build_program(n_tiles, dt_name, general_ln1=False, general_ln2=False):
    """Build the per-core Bass program for n_tiles tiles of 128 samples."""
    dt = _my_dt(dt_name)
    f32 = mybir.dt.float32
    FRelu = mybir.ActivationFunctionType.Relu
    FSqrt = mybir.ActivationFunctionType.Sqrt
    mult = mybir.AluOpType.mult
    add = mybir.AluOpType.add
    sub = mybir.AluOpType.subtract
    amax = mybir.AluOpType.max

    n_rows = n_tiles * P

    nc = bacc.Bacc("TRN2", target_bir_lowering=False, debug=False,
                   num_devices=N_CORES)

    xT = nc.dram_tensor("xT", [K1, n_rows], dt, kind="ExternalInput").ap()
    w1 = nc.dram_tensor("w1", [K1, F1], dt, kind="ExternalInput").ap()
    w2a = nc.dram_tensor("w2a", [F1A, F2], dt, kind="ExternalInput").ap()
    w2b = nc.dram_tensor("w2b", [F1B, F2], dt, kind="ExternalInput").ap()
    ident = nc.dram_tensor("ident", [P, P], dt, kind="ExternalInput").ap()
    if general_ln1:
        g1t = nc.dram_tensor("g1t", [P, F1], dt, kind="ExternalInput").ap()
        b1t = nc.dram_tensor("b1t", [P, F1], dt, kind="ExternalInput").ap()
    if general_ln2:
        g2t = nc.dram_tensor("g2t", [P, F2], f32, kind="ExternalInput").ap()
        b2t = nc.dram_tensor("b2t", [P, F2], f32, kind="ExternalInput").ap()
    out = nc.dram_tensor("out", [n_rows, F2], f32, kind="ExternalOutput").ap()
    # view rows as (tile, partition)
    out_r = out.rearrange("(t p) f -> p t f", p=P)

    with tile.TileContext(nc) as tc:
        with (
            tc.tile_pool(name="consts", bufs=1) as cpool,
            tc.tile_pool(name="xc", bufs=2) as xpool,
            tc.tile_pool(name="psum1", bufs=3, space="PSUM") as p1pool,
            tc.tile_pool(name="hr", bufs=6) as hrpool,
            tc.tile_pool(name="stats", bufs=2) as stpool,
            tc.tile_pool(name="u", bufs=6) as upool,
            tc.tile_pool(name="psumT", bufs=2, space="PSUM") as pTpool,
            tc.tile_pool(name="uT", bufs=6) as uTpool,
            tc.tile_pool(name="psum2", bufs=3, space="PSUM") as p2pool,
            tc.tile_pool(name="st2", bufs=3) as st2pool,
            tc.tile_pool(name="outb", bufs=2) as opool,
        ):
            # --- persistent constants ---
            w1_t = cpool.tile([K1, F1], dt, tag="w1")
            nc.sync.dma_start(w1_t[:], w1)
            w2a_t = cpool.tile([F1A, F2], dt, tag="w2a")
            nc.sync.dma_start(w2a_t[:], w2a)
            # w2b lives at partitions 64:128 to match mm2b's lhsT base
            w2b_t = cpool.tile([P, F2], dt, tag="w2b")
            nc.sync.dma_start(w2b_t[P - F1B:P, :], w2b)
            id_t = cpool.tile([P, P], dt, tag="ident")
            nc.sync.dma_start(id_t[:], ident)
            if general_ln1:
                g1_t = cpool.tile([P, F1], dt, tag="g1t")
                nc.sync.dma_start(g1_t[:], g1t)
                b1_t = cpool.tile([P, F1], dt, tag="b1t")
                nc.sync.dma_start(b1_t[:], b1t)
            if general_ln2:
                g2_t = cpool.tile([P, F2], f32, tag="g2t")
                nc.sync.dma_start(g2_t[:], g2t)
                b2_t = cpool.tile([P, F2], f32, tag="b2t")
                nc.sync.dma_start(b2_t[:], b2t)

            for sg0, sg_n in _iter_chunks(n_tiles, SG):
                # --- load x chunk: [95, sg_n*128] ---
                xc = xpool.tile([K1, SG * P], dt, tag="xc")
                nc.sync.dma_start(xc[:, 0:sg_n * P],
                                  xT[:, sg0 * P:(sg0 + sg_n) * P])

                # --- mm1 + relu + LN1 stats over groups of G1 tiles ---
                hrs = []       # (hr_tile, local offset) per tile
                mv1 = stpool.tile([P, 2 * SG], f32, tag="mv1")  # (mean,var)*SG
                for g0, g_n in _iter_chunks(sg_n, G1):
                    p1 = p1pool.tile([P, 512], f32, tag="p1")
                    for i in range(g_n):
                        t = sg0 + g0 + i
                        nc.tensor.matmul(
                            p1[:, i * F1:(i + 1) * F1],
                            lhsT=xc[:, (g0 + i) * P:(g0 + i + 1) * P],
                            rhs=w1_t[:],
                            start=True, stop=True,
                        )
                    hr = hrpool.tile([P, G1 * F1], dt, tag="hr")
                    nc.scalar.activation(hr[:, 0:g_n * F1], p1[:, 0:g_n * F1],
                                         FRelu)
    # per-tile bn_stats (mean/var in one DVE pass, no accumulator read)
                    bn = stpool.tile([P, G1 * 6], f32, tag="bn1")
                    for i in range(g_n):
                        nc.vector.bn_stats(bn[:, 6 * i:6 * i + 6],
                                           hr[:, i * F1:(i + 1) * F1])
                        nc.vector.bn_aggr(
                            mv1[:, 2 * (g0 + i):2 * (g0 + i) + 2],
                            bn[:, 6 * i:6 * i + 6])
                        hrs.append((hr, i * F1))

                # --- batched LN1 rstd math over the supergroup ---
                mu_v = mv1[:, 0:2 * sg_n].rearrange("p (t two) -> p t two",
                                                    two=2)[:, :, 0]
                var_v = mv1[:, 0:2 * sg_n].rearrange("p (t two) -> p t two",
                                                     two=2)[:, :, 1]
                veps = stpool.tile([P, SG], f32, tag="veps")
                nc.vector.tensor_scalar(veps[:, 0:sg_n], var_v, 1.0, EPS,
                                        mult, op1=add)
                rec = stpool.tile([P, SG], f32, tag="rec")
                nc.vector.reciprocal(rec[:, 0:sg_n], veps[:, 0:sg_n])
                rstd = stpool.tile([P, SG], f32, tag="rstd")
                nc.scalar.activation(rstd[:, 0:sg_n], rec[:, 0:sg_n], FSqrt)
                musr = stpool.tile([P, SG], f32, tag="musr")
                nc.vector.tensor_tensor(musr[:, 0:sg_n], mu_v,
                                        rstd[:, 0:sg_n], mult)

                # --- per tile: affine, transpose, mm2, LN2 ---
                outb = opool.tile([P, SG, F2], f32, tag="outb")

                def flush_ln2(batch, outb=outb):
                    """rstd2 for a batch of tiles + emit finals."""
                    if not batch:
                        return
                    # gather var2 + eps per group (vars are at odd columns)
                    v2 = st2pool.tile([P, LN2_BATCH], f32, tag="v2")
                    done = set()
                    k = 0
                    for (p2, slot, i, mv2) in batch:
                        if id(mv2) in done:
                            continue
                        done.add(id(mv2))
                        n_in_g = sum(1 for b in batch if b[3] is mv2)
                        var_view = mv2[:, 0:2 * n_in_g].rearrange(
                            "p (t two) -> p t two", two=2)[:, :, 1]
                        nc.vector.tensor_scalar(v2[:, k:k + n_in_g], var_view,
                                                1.0, EPS, mult, op1=add)
                        k += n_in_g
                    rec2 = st2pool.tile([P, LN2_BATCH], f32, tag="rec2")
                    nc.vector.reciprocal(rec2[:, 0:k], v2[:, 0:k])
                    rstd2 = st2pool.tile([P, LN2_BATCH], f32, tag="rstd2")
                    nc.scalar.activation(rstd2[:, 0:k], rec2[:, 0:k], FSqrt)
                    for j, (p2, slot, i, mv2) in enumerate(batch):
                        psl = p2[:, slot * F2:(slot + 1) * F2]
                        if general_ln2:
                            tmp = st2pool.tile([P, F2], f32, tag="tmp2")
                            nc.scalar.activation(
                                tmp[:], psl, mybir.ActivationFunctionType.Copy,
                                scale=rstd2[:, j:j + 1])
                            tmp2 = st2pool.tile([P, F2], f32, tag="tmp3")
                            nc.vector.tensor_tensor(tmp2[:], tmp[:], g2_t[:],
                                                    mult)
                            nc.vector.tensor_tensor(tmp[:], tmp2[:], b2_t[:],
                                                    add)
                            nc.vector.tensor_scalar(
                                outb[:, i, :], tmp[:], 0.0, None, amax)
                        else:
                            # final: relu(h2c * rstd2), alternate engines
                            if i % 2 == 0:
                                nc.scalar.activation(
                                    outb[:, i, :], psl, FRelu,
                                    scale=rstd2[:, j:j + 1])
                            else:
                                nc.vector.tensor_scalar(
                                    outb[:, i, :], psl, rstd2[:, j:j + 1],
                                    0.0, mult, op1=amax)

                ln2_batch = []
                p2 = None
                mv2 = None
                for i, (hr, off) in enumerate(hrs):
                    # u = (hr - mu) * rstd  (fused dual-op)
                    u = upool.tile([P, F1 + 1], dt, tag="u")
                    if general_ln1:
                        za = upool.tile([P, F1], dt, tag="za")
                        nc.vector.tensor_scalar(
                            za[:], hr[:, off:off + F1], rstd[:, i:i + 1],
                            musr[:, i:i + 1], mult, op1=sub)
                        zb = upool.tile([P, F1], dt, tag="zb")
                        nc.vector.tensor_tensor(zb[:], za[:], g1_t[:], mult)
                        nc.vector.tensor_tensor(u[:, 0:F1], zb[:], b1_t[:],
                                                add)
                    else:
                        nc.vector.tensor_scalar(
                            u[:, 0:F1], hr[:, off:off + F1], rstd[:, i:i + 1],
                            musr[:, i:i + 1], mult, op1=sub)
                    nc.vector.memset(u[:, F1:F1 + 1], 1.0)

                    # transpose u -> uT (two chunks), relu folded into copy
                    pT = pTpool.tile([P, 2 * P], dt, tag="pT")
                    nc.tensor.transpose(pT[:, 0:P], u[:, 0:F1A], id_t[:])
                    nc.tensor.transpose(pT[:, P:2 * P],
                                        u[:, T2_LO:F1 + 1], id_t[:])
                    uTt = uTpool.tile([P, 2 * P], dt, tag="uT")
                    # single merged relu-copy (relu(1)=1 keeps the ones row)
                    nc.vector.tensor_scalar(uTt[:], pT[:], 0.0, None, amax)

                    # mm2: two accumulating matmuls into a shared PSUM tile
                    slot = i % G2
                    if slot == 0:
                        p2 = p2pool.tile([P, G2 * F2], f32, tag="p2")
                        mv2 = st2pool.tile([P, 2 * G2], f32, tag="mv2")
                    psl = p2[:, slot * F2:(slot + 1) * F2]
                    nc.tensor.matmul(psl, lhsT=uTt[:, 0:P], rhs=w2a_t[:],
                                     start=True, stop=False)
                    nc.tensor.matmul(psl, lhsT=uTt[P - F1B:P, P:2 * P],
                                     rhs=w2b_t[P - F1B:P, :],
                                     start=False, stop=True)

                    ln2_batch.append((p2, slot, i, mv2))
                    bn2 = st2pool.tile([P, 6], f32, tag="bn2")
                    nc.vector.bn_stats(bn2[:], psl)
                    nc.vector.bn_aggr(mv2[:, 2 * slot:2 * slot + 2], bn2[:])
                    if len(ln2_batch) == LN2_BATCH:
                        flush_ln2(ln2_batch)
                        ln2_batch = []
                flush_ln2(ln2_batch)

                # --- store supergroup output ---
                nc.sync.dma_start(out_r[:, sg0:sg0 + sg_n, :],
                                  outb[:, 0:sg_n, :])

    nc.compile()
    return nc


def _prep_host(inputs, dt_name):
    """Fold weights, transpose/augment x; returns per-core input maps."""
    ndt = _np_dt(dt_name)
    x = np.asarray(inputs["x"], np.float32)
    assert x.shape == (B_TOTAL, 94), x.shape

    # W1 [95, 160]: block-diagonal branch weights + bias row
    w1 = np.zeros((K1, F1), np.float32)
    for name, il, ih, ol, oh in _BRANCHES:
        w1[il:ih, ol:oh] = np.asarray(inputs[f"w_{name}"], np.float32)
        w1[94, ol:oh] = np.asarray(inputs[f"b_{name}"], np.float32)

    # LN params
    ln1_g = np.asarray(inputs["ln1_g"], np.float32)
    ln1_b = np.asarray(inputs["ln1_b"], np.float32)
    ln2_g = np.asarray(inputs["ln2_g"], np.float32)
    ln2_b = np.asarray(inputs["ln2_b"], np.float32)
    general_ln1 = not (np.allclose(ln1_g, 1.0) and np.allclose(ln1_b, 0.0))
    general_ln2 = not (np.allclose(ln2_g, 1.0) and np.allclose(ln2_b, 0.0))

    # W2 [161, 128]: row-centered w_fuse + centered bias row
    wf = np.asarray(inputs["w_fuse"], np.float32)
    bf = np.asarray(inputs["b_fuse"], np.float32)
    wc = wf - wf.mean(axis=1, keepdims=True)
    bc = bf - bf.mean()
    w2 = np.concatenate([wc, bc[None, :]], axis=0)  # [161, 128]
    # mm2b lhsT rows map to u cols 97:161 (features 97:160 + ones); features
    # 97:128 are already covered by mm2a, so those rows are zero.
    w2b = np.zeros((F1B, F2), np.float32)
    w2b[F1B - 33:F1B] = w2[F1A:F1 + 1]

    # xT augmented with ones row: [95, B]
    xT = np.empty((K1, B_TOTAL), np.float32)
    xT[0:94] = x.T
    xT[94] = 1.0

    ident = np.eye(P, dtype=np.float32)

    core_maps = []
    for c in range(N_CORES):
        m = {
            "xT": np.ascontiguousarray(
                xT[:, c * B_CORE:(c + 1) * B_CORE]).astype(ndt),
            "w1": w1.astype(ndt),
            "w2a": np.ascontiguousarray(w2[0:F1A]).astype(ndt),
            "w2b": w2b.astype(ndt),
            "ident": ident.astype(ndt),
        }
        if general_ln1:
            m["g1t"] = np.tile(ln1_g[None, :], (P, 1)).astype(ndt)
            m["b1t"] = np.tile(ln1_b[None, :], (P, 1)).astype(ndt)
        if general_ln2:
            m["g2t"] = np.tile(ln2_g[None, :], (P, 1)).astype(np.float32)
            m["b2t"] = np.tile(ln2_b[None, :], (P, 1)).astype(np.float32)
        core_maps.append(m)
    return core_maps, general_ln1, general_ln2


def kernel(**inputs):
    global LAST_RESULTS
    core_maps, gl1, gl2 = _prep_host(inputs, DT_NAME)
    key = (DT_NAME, B_CORE // P, gl1, gl2)
    if key not in _PROGRAM_CACHE:
        _PROGRAM_CACHE[key] = build_program(B_CORE // P, DT_NAME, gl1, gl2)
    nc = _PROGRAM_CACHE[key]

    res = run_bass_kernel_spmd(nc, core_maps, list(range(N_CORES)),
                               trace=TRACE)
    LAST_RESULTS = res
    out = np.empty((B_TOTAL, F2), np.float32)
    for c in range(N_CORES):
        out[c * B_CORE:(c + 1) * B_CORE] = res.results[c]["out"]
    return out

